# revision 75
# baseline (speedup 1.0000x reference)
"""GQA attention (RoPE + ALiBi + causal) on 8 trn2 NeuronCores.

Sharding: core c -> batch b = c//4, kv-group g = c%4 (4 q-heads + 1 kv-head
per core, column-sharded Wq/Wk/Wv, row-sharded Wo; host sums the 4 partial
Wo outputs per batch).

v2: software-pipelined phase emission (proj A / attention B / out-proj C
interleaved A0,A1,B0,A2,B1,C0,A3,B2,C1,B3,C2,C3) so the PE instruction
queue always has runnable matmuls and the HAM clock gate stays warm.
All matmuls run in bf16 (same 1 cyc/row PE rate as fp32r, half the DMA/
SBUF/LDWEIGHTS cost). V is produced directly transposed by using the x
tiles as the stationary operand. Softmax scale rides the exp activation's
`scale` (mask pre-divided by it on host); per-key ALiBi rides the exp
bias; the per-query ALiBi residual cancels in softmax. Normalization uses
reciprocal_approx_fast. Score/exp/reduce emission is staggered so the PE
never waits on the scalar engine's exp.
"""
import sys

if '/opt/trn_rl_repo' not in sys.path:
    sys.path.insert(0, '/opt/trn_rl_repo')

import numpy as np
import ml_dtypes

BF = ml_dtypes.bfloat16

B, T, D = 2, 2048, 2048
H, KV = 16, 4
HD = D // H          # 128
NREP = H // KV       # 4
KVD = 512            # per-core q width (4 heads x 128)
P = 128
TB = 512             # t-block
NBLK = T // TB       # 4
NC = D // P          # 16 contraction tiles
NJ = T // P          # 16 key tiles
ALIBI_W = 0.1
SCALE = float((1.0 - ALIBI_W) / np.sqrt(np.float32(HD)))

_cache = {}


def _build():
    from concourse import bacc, mybir
    from concourse.tile import TileContext

    F32 = mybir.dt.float32
    BF16 = mybir.dt.bfloat16
    EXP = mybir.ActivationFunctionType.Exp

    nc = bacc.Bacc()
    xT = nc.declare_dram_parameter("xT", [D, T], BF16, isOutput=False)
    wq = nc.declare_dram_parameter("wq", [D, KVD], BF16, isOutput=False)
    wk = nc.declare_dram_parameter("wk", [D, P], BF16, isOutput=False)
    wv = nc.declare_dram_parameter("wv", [D, P], BF16, isOutput=False)
    wo = nc.declare_dram_parameter("wo", [KVD, D], BF16, isOutput=False)
    cosT = nc.declare_dram_parameter("cosT", [P, T], BF16, isOutput=False)
    sinT = nc.declare_dram_parameter("sinT", [P, T], BF16, isOutput=False)
    cb = nc.declare_dram_parameter("cb", [P, NREP * NBLK * NJ], F32, isOutput=False)
    maskT = nc.declare_dram_parameter("maskT", [P, P], BF16, isOutput=False)
    onesc = nc.declare_dram_parameter("onesc", [P, 1], BF16, isOutput=False)
    permm = nc.declare_dram_parameter("permm", [P, P], BF16, isOutput=False)
    out = nc.declare_dram_parameter("out", [T, D], BF16, isOutput=True)

    with TileContext(nc) as tc:
        with (
            tc.tile_pool(name="const", bufs=1) as cpool,
            tc.tile_pool(name="kv", bufs=1) as kvpool,
            tc.tile_pool(name="xin", bufs=36) as xpool,
            tc.tile_pool(name="rope", bufs=3) as rpool,
            tc.tile_pool(name="qt", bufs=12) as qpool,
            tc.tile_pool(name="pt", bufs=14) as ptpool,
            tc.tile_pool(name="oh", bufs=8) as opool,
            tc.tile_pool(name="ysb", bufs=4) as ypool,
            tc.tile_pool(name="small", bufs=2) as spool,
            tc.tile_pool(name="ps", bufs=1, space="PSUM") as pss,
        ):
            # ---- constants; DMA issue order is tuned so block-0 x tiles
            # interleave with the weights the first matmuls actually need,
            # and late-phase constants (mask/cb/wo) queue behind ----
            wq_sb = cpool.tile([P, NC, KVD], BF16)
            wq_r = wq.rearrange("(c p) n -> p c n", p=P)
            wk_sb = cpool.tile([P, NC, P], BF16)
            wk_r = wk.rearrange("(c p) n -> p c n", p=P)
            wv_sb = cpool.tile([P, NC, P], BF16)
            wv_r = wv.rearrange("(c p) n -> p c n", p=P)
            wo_sb = cpool.tile([P, NREP, D], BF16)
            wo_r = wo.rearrange("(h p) e -> p h e", p=P)
            # weights/tables ride the Activation-engine DMA ring; the x
            # stream and output writes ride the sync-engine ring, so the
            # two hardware DGE queues transfer in parallel.
            x0ts = []
            for c in range(NC):
                nc.scalar.dma_start(out=wq_sb[:, c], in_=wq_r[:, c])
                xt = xpool.tile([P, TB], BF16, tag="xt", name=f"xt0_{c}")
                nc.sync.dma_start(out=xt, in_=xT[c * P:(c + 1) * P, 0:TB])
                x0ts.append(xt)
            # k/v weights are first needed at the kv-pass, half a segment
            # after the q-pass: load them behind x0 on the x ring
            for c4 in range(4):
                nc.sync.dma_start(out=wk_sb[:, c4 * 4:(c4 + 1) * 4],
                                  in_=wk_r[:, c4 * 4:(c4 + 1) * 4])
                nc.sync.dma_start(out=wv_sb[:, c4 * 4:(c4 + 1) * 4],
                                  in_=wv_r[:, c4 * 4:(c4 + 1) * 4])
            cos_sb = cpool.tile([P, T], BF16)
            nc.scalar.dma_start(out=cos_sb, in_=cosT[:, :])
            sin_sb = cpool.tile([P, T], BF16)
            nc.scalar.dma_start(out=sin_sb, in_=sinT[:, :])
            onesc_sb = cpool.tile([P, 1], BF16)
            nc.scalar.dma_start(out=onesc_sb, in_=onesc[:, :])
            perm_sb = cpool.tile([P, P], BF16)
            nc.scalar.dma_start(out=perm_sb, in_=permm[:, :])
            x1ts = []
            for c in range(NC):
                xt = xpool.tile([P, TB], BF16, tag="xt", name=f"xt1_{c}")
                nc.sync.dma_start(out=xt, in_=xT[c * P:(c + 1) * P, TB:2 * TB])
                x1ts.append(xt)
            cb_sb = cpool.tile([P, NREP * NBLK * NJ], F32)
            tri_sb = cpool.tile([P, P], BF16)

            def load_late_consts():
                nc.scalar.dma_start(out=cb_sb, in_=cb[:, :])
                nc.scalar.dma_start(out=tri_sb, in_=maskT[:, :])

            def load_wo():
                for h in range(NREP):
                    nc.sync.dma_start(out=wo_sb[:, h], in_=wo_r[:, h])

            # roped K, [d, s]: one tile per t-block so a block's k-rope
            # write never carries a WAR dependency on the previous
            # attention segment's reads
            kT_sb = [kvpool.tile([P, TB], BF16, name=f"kTsb{b_}")
                     for b_ in range(NBLK)]
            v_sb = kvpool.tile([P, NJ * P], BF16)    # V transposed, [s_local, j*128+d']

            q_sb = {}    # (bk, h) -> [d', t] bf16 roped q
            oh_sb = {}   # (bk, h) -> [d', t] bf16 normalized attention out

            def rope_pre(src_ps, nm):
                # PSUM-freeing raw copy; emitted immediately after the
                # projection c-loop so the PSUM banks recycle fast
                raw = rpool.tile([P, TB], BF16, tag="raw", bufs=7, name=f"raw{nm}")
                nc.vector.tensor_copy(raw, src_ps)
                return raw

            def rope_post(dst, raw, t0, nm):
                # partition half-swap via a permutation matmul on the PE
                # (no DMA: keeps the sync queue free of cross-engine waits)
                m1 = rpool.tile([P, TB], F32, tag="m1", name=f"m1{nm}")
                nc.vector.tensor_mul(m1, raw, cos_sb[:, t0:t0 + TB])
                sw_ps = pss.tile([P, TB], F32, tag="big", bufs=6, name=f"swps{nm}")
                nc.tensor.matmul(sw_ps, perm_sb, raw, start=True, stop=True)
                m2 = rpool.tile([P, TB], F32, tag="m2", name=f"m2{nm}")
                nc.vector.tensor_mul(m2, sw_ps, sin_sb[:, t0:t0 + TB])
                nc.vector.tensor_add(dst, m1, m2)

            rope_state = {}
            a_state = {}

            def phase_a_q(bk, xts=None):
                """q projections for t-block bk + their PSUM-freeing raws"""
                t0 = bk * TB
                q_ps = [pss.tile([P, TB], F32, tag="big", bufs=6, name=f"qps{bk}_{h}")
                        for h in range(NREP)]
                all_xts = []
                for c in range(NC):
                    if xts is not None:
                        xt = xts[c]
                    else:
                        xt = xpool.tile([P, TB], BF16, tag="xt", name=f"xt{bk}_{c}")
                        nc.sync.dma_start(out=xt, in_=xT[c * P:(c + 1) * P, t0:t0 + TB])
                    all_xts.append(xt)
                    for h in range(NREP):
                        nc.tensor.matmul(q_ps[h], wq_sb[:, c, h * P:(h + 1) * P], xt,
                                         start=(c == 0), stop=(c == NC - 1))
                heads = [rope_pre(q_ps[h], f"q{bk}_{h}") for h in range(NREP)]
                a_state[bk] = (all_xts, heads)

            def phase_a_kv(bk):
                """k/v projections; banks allocate late so they rotate onto
                PSUM slots the interleaved B segment has already freed"""
                all_xts, heads = a_state.pop(bk)
                k_ps = pss.tile([P, TB], F32, tag="big", bufs=6, name=f"kps{bk}")
                v_ps = pss.tile([P, TB], F32, tag="big", bufs=6, name=f"vps{bk}")
                for c in range(NC):
                    xt = all_xts[c]
                    nc.tensor.matmul(k_ps, wk_sb[:, c, :], xt,
                                     start=(c == 0), stop=(c == NC - 1))
                    nc.tensor.matmul(v_ps, wv_sb[:, c, :], xt,
                                     start=(c == 0), stop=(c == NC - 1))
                heads.append(rope_pre(k_ps, f"k{bk}"))
                vtmp = rpool.tile([P, TB], BF16, tag="vtmp", name=f"vtmp{bk}")
                nc.scalar.copy(vtmp, v_ps)
                rope_state[bk] = (heads, vtmp)

            def phase_a(bk, xts=None):
                phase_a_q(bk, xts)
                phase_a_kv(bk)

            def phase_a_post(bk):
                """rope tails + V transposes; emitted after the next B segment
                so its DVE ops sit behind that segment's critical masks"""
                t0 = bk * TB
                heads, vtmp = rope_state.pop(bk)
                for h in range(NREP):
                    qh = qpool.tile([P, TB], BF16, tag="qT", name=f"qT{bk}_{h}")
                    rope_post(qh, heads[h], t0, f"q{bk}_{h}")
                    q_sb[(bk, h)] = qh
                rope_post(kT_sb[bk], heads[4], t0, f"k{bk}")
                for ts_ in range(4):
                    j = 4 * bk + ts_
                    nc.sync.dma_start(out=v_sb[:, j * P:(j + 1) * P],
                                      in_=vtmp[:, ts_ * P:(ts_ + 1) * P],
                                      transpose=True)

            def phase_b(bk, hs=None, after_unit=None):
                """attention for q-block bk over key tiles 0..4*bk+3"""
                nj = 4 * bk + 4
                if hs is None:
                    hs = list(range(NREP))

                def make_unit(h):
                    q = q_sb[(bk, h)]
                    pts = {}
                    acc = {}

                    def emit_s(j):
                        # diagonal key tile (delta>=0): queries below the
                        # triangle chunk are fully masked -> skip those
                        # columns outright; only the [128,128] chunk at
                        # column delta*128 needs the triangular 0/1 mask
                        delta = j - 4 * bk
                        lo = max(0, delta) * P
                        s_ps = pss.tile([P, TB], F32, tag="big", bufs=6,
                                        name=f"sps{bk}_{h}_{j}")
                        nc.tensor.matmul(s_ps[:, lo:TB],
                                         kT_sb[j // 4][:, (j % 4) * P:(j % 4 + 1) * P],
                                         q[:, lo:TB], start=True, stop=True)
                        pt = ptpool.tile([P, TB], BF16, tag="pt", name=f"pt{bk}_{h}_{j}")
                        col = (h * NBLK + bk) * NJ + j
                        nc.scalar.activation(pt[:, lo:TB], s_ps[:, lo:TB], EXP,
                                             bias=cb_sb[:, col:col + 1], scale=SCALE)
                        if delta >= 0:
                            nc.vector.tensor_mul(pt[:, lo:lo + P], pt[:, lo:lo + P],
                                                 tri_sb)
                        pts[j] = (pt, lo)

                    def emit_red(j):
                        if j == 0:
                            acc['ot'] = pss.tile([P, TB], F32, tag="big", bufs=6,
                                                 name=f"otps{bk}_{h}")
                            acc['cs'] = pss.tile([1, TB], F32, tag="cs", bufs=2,
                                                 name=f"csps{bk}_{h}")
                        pt, lo = pts[j]
                        nc.tensor.matmul(acc['cs'][:, lo:TB], onesc_sb, pt[:, lo:TB],
                                         start=(j == 0), stop=(j == nj - 1))
                        nc.tensor.matmul(acc['ot'][:, lo:TB],
                                         v_sb[:, j * P:(j + 1) * P], pt[:, lo:TB],
                                         start=(j == 0), stop=(j == nj - 1))

                    def norm():
                        rec = spool.tile([1, TB], F32, tag="rec", name=f"rec{bk}_{h}")
                        nc.vector.reciprocal_approx_fast(rec, acc['cs'])
                        rbc = spool.tile([P, TB], F32, tag="rbc", name=f"rbc{bk}_{h}")
                        nc.gpsimd.partition_broadcast(rbc, rec)
                        oh = opool.tile([P, TB], BF16, tag="oh", name=f"oh{bk}_{h}")
                        nc.vector.tensor_mul(oh, acc['ot'], rbc)
                        oh_sb[(bk, h)] = oh

                    return emit_s, emit_red, norm

                if bk == 0:
                    # all tiles are diagonal: front-load the units' score
                    # streams so the PE has work while the DVE catches up
                    # on the rope raws + triangular masks
                    units = {h: make_unit(h) for h in hs}
                    for h in hs:
                        for j in range(nj):
                            units[h][0](j)
                    for h in hs:
                        for j in range(nj):
                            units[h][1](j)
                        units[h][2]()
                else:
                    for h in hs:
                        emit_s, emit_red, norm = make_unit(h)
                        stag = 5 if nj >= 8 else min(4, nj)
                        for j in range(stag):
                            emit_s(j)
                        for j in range(nj):
                            if j + stag < nj:
                                emit_s(j + stag)
                            emit_red(j)
                        norm()
                        if after_unit is not None:
                            after_unit(h)

            def phase_c_unit(bk, ts_, e, use_act=False):
                t0 = bk * TB
                y_ps = pss.tile([P, TB], F32, tag="big", bufs=6,
                                name=f"yps{bk}_{ts_}_{e}")
                for h in range(NREP):
                    nc.tensor.matmul(y_ps,
                                     oh_sb[(bk, h)][:, ts_ * P:(ts_ + 1) * P],
                                     wo_sb[:, h, e * TB:(e + 1) * TB],
                                     start=(h == 0), stop=(h == NREP - 1))
                y_sb = ypool.tile([P, TB], BF16, tag="ysb", name=f"y{bk}_{ts_}_{e}")
                if use_act:
                    nc.scalar.copy(y_sb, y_ps)
                    nc.scalar.dma_start(
                        out=out[t0 + ts_ * P:t0 + (ts_ + 1) * P, e * TB:(e + 1) * TB],
                        in_=y_sb)
                else:
                    nc.vector.tensor_copy(y_sb, y_ps)
                    nc.sync.dma_start(
                        out=out[t0 + ts_ * P:t0 + (ts_ + 1) * P, e * TB:(e + 1) * TB],
                        in_=y_sb)

            def phase_c(bk, split_engines=False):
                """output projection partial for t-block bk"""
                for ts_ in range(4):
                    for e in range(4):
                        phase_c_unit(bk, ts_, e,
                                     use_act=(split_engines and e % 2 == 1))

            # software-pipelined emission: keep the PE queue dense across phases
            phase_a(0, xts=x0ts)
            phase_a_post(0)
            load_late_consts()
            phase_a(1, xts=x1ts)
            phase_b(0)
            phase_a_post(1)
            phase_a(2)
            phase_b(1)
            phase_a_post(2)
            load_wo()
            phase_c(0)
            phase_a(3)
            phase_b(2)
            phase_a_post(3)
            phase_c(1)
            phase_b(3)
            phase_c(2, split_engines=True)
            phase_c(3, split_engines=True)

    nc.compile()
    return nc


def _prep_inputs(x, mask, freqs_cis, alibi_bias, Wq, Wk, Wv, Wo):
    """Host-side prep: transposes, RoPE tables, ALiBi bias decomposition."""
    f64 = np.float64
    idx = np.arange(HD)
    cos_full = freqs_cis[:, idx // 2]                     # [T, 128]
    sin_full = freqs_cis[:, (HD // 2) + idx // 2]         # [T, 128]
    sign = np.where(idx < HD // 2, -1.0, 1.0).astype(np.float32)
    cosT = np.ascontiguousarray(cos_full.T).astype(BF)                # [128, T]
    sinT = np.ascontiguousarray((sin_full * sign[None, :]).T).astype(BF)

    # 0/1 lower-tri keep-mask (key s_local <= query t_local), applied
    # post-exp to the single triangular 128x128 chunk of diagonal tiles
    sl = np.arange(P)
    maskT = (sl[:, None] <= sl[None, :]).astype(BF)

    onesc = np.ones((P, 1), BF)
    # lhsT for the half-swap: out[m] = raw[(m+64)%128] -> lhsT[k,m]=1 iff k=(m+64)%128
    permm = np.zeros((P, P), np.float32)
    permm[(np.arange(P) + 64) % P, np.arange(P)] = 1.0
    permm = permm.astype(BF)

    in_maps = []
    for c in range(8):
        b, g = c // 4, c % 4
        slopes = np.array([-f64(alibi_bias[0, g * NREP + hl, 1, 0]) for hl in range(NREP)])
        pvec = np.arange(P, dtype=f64)
        jvec = np.arange(NJ, dtype=f64)
        # cb[p, h, bk, j] = ALIBI_W*slope*(j*128 + p) - ALIBI_W*slope*(bk*512 + 511)
        bkvec = np.arange(NBLK, dtype=f64)
        cbv = (ALIBI_W * slopes[:, None, None, None]
               * (jvec[None, None, :, None] * P + pvec[None, None, None, :]
                  - (bkvec[None, :, None, None] * TB + (TB - 1))))
        cbm = np.ascontiguousarray(cbv.transpose(3, 0, 1, 2).reshape(P, NREP * NBLK * NJ)).astype(np.float32)
        in_maps.append({
            "xT": np.ascontiguousarray(x[b].T).astype(BF),
            "wq": np.ascontiguousarray(Wq[:, g * KVD:(g + 1) * KVD]).astype(BF),
            "wk": np.ascontiguousarray(Wk[:, g * P:(g + 1) * P]).astype(BF),
            "wv": np.ascontiguousarray(Wv[:, g * P:(g + 1) * P]).astype(BF),
            "wo": np.ascontiguousarray(Wo[g * KVD:(g + 1) * KVD, :]).astype(BF),
            "cosT": cosT, "sinT": sinT,
            "cb": cbm, "maskT": maskT,
            "onesc": onesc, "permm": permm,
        })
    return in_maps


def kernel(x, mask, freqs_cis, alibi_bias, Wq, Wk, Wv, Wo, _trace=False, _trace_kwargs=None):
    from concourse.bass_utils import run_bass_kernel_spmd

    if "nc" not in _cache:
        _cache["nc"] = _build()
    nc = _cache["nc"]

    in_maps = _prep_inputs(np.asarray(x, np.float32), np.asarray(mask, np.float32),
                           np.asarray(freqs_cis, np.float32), np.asarray(alibi_bias, np.float32),
                           np.asarray(Wq, np.float32), np.asarray(Wk, np.float32),
                           np.asarray(Wv, np.float32), np.asarray(Wo, np.float32))
    kw = {}
    if _trace:
        kw = dict(trace=True, **(_trace_kwargs or {}))
    res = run_bass_kernel_spmd(nc, in_maps, list(range(8)), **kw)

    full = np.zeros((B, T, D), np.float32)
    for c in range(8):
        full[c // 4] += res.results[c]["out"].astype(np.float32)
    if _trace:
        _cache["last_trace"] = res
    return full
